# revision 3
# baseline (speedup 1.0000x reference)
"""DAS (delay-and-sum) beamforming on 8 Trainium2 NeuronCores.

Strategy: PE-matmul delay-block gather (replaces the GPSIMD indirect_copy
baseline, ~3x faster).

The delay index map t(u,w) = int(sqrt((u*DX)^2+(w*DY)^2)/VS/DT) is constant
geometry; a sensor at (x,y) merely windows it.  Pair := (sensor s, image
column ixi); U := x_s - ixi is shared across pairs, so for each U a constant
one-hot "delay block" B_U[128, 1024] (one-hot over tau mod 128 per rho
column, rho = 511 - w) turns the per-pixel gather into PE matmuls:

    psum[(q,c), rho] = sum_tau' data[tau', (s_q, chunk, c)] * B_U[tau', rho]

with the 2048-sample contraction split into 16 tau-chunks of 128; each rho
column belongs to exactly one chunk, so per (U, chunk-interval) a single
matmul (stationary = data slice [128, 16 sensors x 8 ch], moving = block
columns) writes a disjoint psum column range.  Pairs are grouped 16-at-a-time
by x-sorted sensor rank so the stationary is one contiguous slice.

Output: each psum row is a 1024-long image-row profile; indirect_dma_start
with compute_op=add scatters all 128 rows of a group into a padded per-core
image in HBM at per-row element offsets (shift by y_s).  DMA-add collisions
race, so same-ixi rows are kept >=2 output instructions apart (depth-2 DMA
pipeline + per-group conflict flags force deeper waits when needed).

Sharding: cores take contiguous U ranges balanced by group count (each core
builds its own program; compiles are cached per sensor_xy).  The host sums
the 8 padded partial images and crops to (4, 2, 512, 512).
"""

import numpy as np

import concourse.bass as bass
import concourse.mybir as mybir

F32 = mybir.dt.float32
BF16 = mybir.dt.bfloat16
I32 = mybir.dt.int32
USE_BF16 = True
MMDT = BF16

NX = NY = 512
DX = DY = 1e-4
VS = 1550.0
DT = 2.5e-8

S = 128
T = 2048
NCHUNK = 16
NU = 1023            # U in [-511, 511]
NRHO = 1024          # rho = 511 - w, w in [-512, 511]
PADW = 1536          # padded image row width (cols y..y+1023, y<=511)
NROW = 8 * 512       # (c, ixi) rows
DUMP_ROW = NROW      # scratch row for invalid pairs
NCORES = 8

# ---------------------------------------------------------------- constants

_CONST = {}


def _tables():
    if "T" in _CONST:
        return _CONST["T"], _CONST["CHUNK"], _CONST["BLOCKS"]
    U = np.arange(-511, 512, dtype=np.float32)[:, None]
    W = (511 - np.arange(NRHO, dtype=np.float32))[None, :]
    dx = (U * np.float32(DX)) ** 2
    dy = (W * np.float32(DY)) ** 2
    dis = np.sqrt(dx + dy, dtype=np.float32)
    tt = (dis / np.float32(VS) / np.float32(DT)).astype(np.int32)  # [1023,1024]
    chunk = (tt >> 7).astype(np.int8)
    blocks = np.zeros((NU, 128, NRHO), np.float32)
    ui = np.broadcast_to(np.arange(NU)[:, None], tt.shape)
    rho = np.broadcast_to(np.arange(NRHO)[None, :], tt.shape)
    blocks[ui, tt & 127, rho] = 1.0
    _CONST["T"] = tt
    _CONST["CHUNK"] = chunk
    _CONST["BLOCKS"] = blocks
    return tt, chunk, blocks


def _segs_for_u(chunkrow):
    """[(chunk, a, b)] covering rho [0,1024), split at 512."""
    segs = []
    a = 0
    cur = int(chunkrow[0])
    for r in range(1, NRHO + 1):
        c = int(chunkrow[r]) if r < NRHO else -1
        if c != cur or r == NRHO:
            # split at the 512 psum-half boundary
            lo, hi = a, r
            if lo < 512 < hi:
                segs.append((cur, lo, 512))
                segs.append((cur, 512, hi))
            else:
                segs.append((cur, lo, hi))
            a, cur = r, c
    return segs


# ---------------------------------------------------------------- planning


def _perm_ranks(x):
    """x-sorted sensor order; spread equal-x sensors >=32 ranks apart."""
    order = list(np.argsort(x, kind="stable"))
    xs = [int(x[o]) for o in order]
    for i in range(S):
        j = 1
        while i + j < S and xs[i + j] == xs[i]:
            # move dup at i+j to i+j+32*j region by swapping
            tgt = i + j + 32 * j
            if tgt >= S:
                tgt = max(0, i - 32 * j)
            order[i + j], order[tgt] = order[tgt], order[i + j]
            xs[i + j], xs[tgt] = xs[tgt], xs[i + j]
            j += 1
    return np.array(order)


def make_plan(sensor_xy):
    tt, chunktab, _ = _tables()
    xy = np.asarray(sensor_xy)
    order = _perm_ranks(xy[:, 0])
    xs = xy[order, 0].astype(np.int64)
    ys = xy[order, 1].astype(np.int64)

    # groups per U
    u_entries = []   # (U, [ (k, valid16) ... ])
    for U in range(-511, 512):
        ixi = xs - U
        valid = (ixi >= 0) & (ixi < 512)
        if not valid.any():
            continue
        vr = np.flatnonzero(valid)
        lo, hi = int(vr[0]), int(vr[-1]) + 1
        ks = []
        prev_end = lo
        for w in range(lo, hi, 16):
            r0c = min(w, 112)
            vmask = np.zeros(16, bool)
            for r in range(max(w, prev_end), min(w + 16, hi)):
                if valid[r]:
                    vmask[r - r0c] = True
            prev_end = min(w + 16, hi)
            if vmask.any():
                ks.append((r0c, vmask))
        u_entries.append((U, ks))

    # split U entries into 8 contiguous runs with ~equal group counts
    costs = np.array([len(ks) + 0.35 for _, ks in u_entries])
    cum = np.cumsum(costs)
    total = cum[-1]
    bounds = [0]
    for c in range(1, NCORES):
        bounds.append(int(np.searchsorted(cum, total * c / NCORES)))
    bounds.append(len(u_entries))

    cores = []
    for ci in range(NCORES):
        ents = u_entries[bounds[ci]:bounds[ci + 1]]
        u_list = []           # U values
        seg_list = []         # segs per U
        groups = []           # (ui, k, off_col[128], ixi_set)
        gend = []             # groups completed after each U
        for U, ks in ents:
            ui = len(u_list)
            u_list.append(U)
            seg_list.append(_segs_for_u(chunktab[U + 511]))
            for r0c, v16 in ks:
                off = ((DUMP_ROW + np.arange(128)) * PADW).astype(np.int64)
                ixis = set()
                for q in range(16):
                    r = r0c + q
                    if v16[q]:
                        ixi = int(xs[r] - U)
                        ixis.add(ixi)
                        for c in range(8):
                            off[q * 8 + c] = (c * 512 + ixi) * PADW + ys[r]
                groups.append((ui, r0c, off.astype(np.int32), ixis))
            gend.append(len(groups))
        # conflict pass: adjacent output instrs must not share ixi
        deep = [False] * len(groups)
        for g in range(1, len(groups)):
            if groups[g][3] & groups[g - 1][3]:
                deep[g] = True
        cores.append(dict(u_list=u_list, seg_list=seg_list, groups=groups,
                          gend=gend, deep=deep))
    return dict(order=order, xs=xs, ys=ys, cores=cores)


# ---------------------------------------------------------------- bass build


def build_core_nc(plan_core, repeat=1, mode="full"):
    u_list = plan_core["u_list"]
    seg_list = plan_core["seg_list"]
    groups = plan_core["groups"]
    gend = plan_core["gend"]
    deep = plan_core["deep"]
    nU = len(u_list)
    G = len(groups)
    GT = repeat * G

    nc = bass.Bass()
    data = nc.declare_dram_parameter("data", [128, NCHUNK * 128 * 8], MMDT,
                                     isOutput=False)
    blocks = nc.declare_dram_parameter("blocks", [nU * 128, NRHO], MMDT,
                                       isOutput=False)
    offs = nc.declare_dram_parameter("offs", [128, G], I32, isOutput=False)
    img = nc.declare_dram_parameter("img", [(NROW + 128) * PADW, 1], F32,
                                    isOutput=True)

    from contextlib import ExitStack
    with ExitStack() as ctx:
        data_sb = ctx.enter_context(
            nc.sbuf_tensor("data_sb", [128, NCHUNK * 128 * 8], MMDT))
        blk = [ctx.enter_context(nc.sbuf_tensor(f"blk{i}", [128, NRHO], MMDT))
               for i in range(2)]
        offs_sb = ctx.enter_context(nc.sbuf_tensor("offs_sb", [128, G], I32))
        stg = [ctx.enter_context(nc.sbuf_tensor(f"stg{i}", [128, NRHO], F32))
               for i in range(4)]
        ps = [[ctx.enter_context(
            nc.psum_tensor(f"ps{i}{h}", [128, 512], F32)) for h in range(2)]
            for i in range(4)]
        d_sem = ctx.enter_context(nc.semaphore("d_sem"))
        b_sem = ctx.enter_context(nc.semaphore("b_sem"))
        m_done = ctx.enter_context(nc.semaphore("m_done"))
        a_done = ctx.enter_context(nc.semaphore("a_done"))
        v_done = ctx.enter_context(nc.semaphore("v_done"))
        o_sem = ctx.enter_context(nc.semaphore("o_sem"))
        block = ctx.enter_context(nc.Block())

        @block.sync
        def _(sync):
            sync.dma_start(data_sb[:, :], data[:, :]).then_inc(d_sem, 16)
            sync.dma_start(offs_sb[:, :], offs[:, :]).then_inc(d_sem, 16)
            for rep in range(repeat):
                for ui in range(nU):
                    ug = rep * nU + ui
                    if ug >= 2:
                        pu = ug - 2
                        pg = (pu // nU) * G + gend[pu % nU]
                        sync.wait_ge(m_done, pg)
                    sync.dma_start(
                        blk[ug % 2][:, :],
                        blocks[ui * 128:(ui + 1) * 128, :],
                    ).then_inc(b_sem, 16)
            if mode in ("full", "out"):
                sync.wait_ge(o_sem, 16 * GT)
            else:
                sync.wait_ge(a_done, GT)
                sync.wait_ge(v_done, GT)

        @block.tensor
        def _(tensor):
            tensor.wait_ge(d_sem, 32)
            g = 0
            for rep in range(repeat):
                for ui in range(nU):
                    ug = rep * nU + ui
                    tensor.wait_ge(b_sem, 16 * (ug + 1))
                    segs = seg_list[ui]
                    g0 = rep * G + gend[ui - 1] if ui > 0 else rep * G
                    g1 = rep * G + gend[ui]
                    for g in range(g0, g1):
                        _, k, _, _ = groups[g % G]
                        if g >= 4:
                            tensor.wait_ge(a_done, g - 3)
                            tensor.wait_ge(v_done, g - 3)
                        if mode == "out":
                            tensor.matmul(
                                ps[g % 4][0][:, 0:4],
                                data_sb[:, k * 128:k * 128 + 128],
                                blk[ug % 2][:, 0:4],
                                start=True, stop=True,
                            ).then_inc(m_done, 1)
                            continue
                        mm = None
                        for (ch, a, b) in segs:
                            half = 1 if a >= 512 else 0
                            mm = tensor.matmul(
                                ps[g % 4][half][:, a - 512 * half:b - 512 * half],
                                data_sb[:, ch * 1024 + k * 8:
                                        ch * 1024 + k * 8 + 128],
                                blk[ug % 2][:, a:b],
                                start=True, stop=True,
                            )
                        mm.then_inc(m_done, 1)

        ncols = 4 if mode == "pe" else 512

        @block.scalar
        def _(scalar):
            for g in range(GT):
                scalar.wait_ge(m_done, g + 1)
                if g >= 4 and mode in ("full", "out"):
                    scalar.wait_ge(o_sem, 16 * (g - 3))
                scalar.copy(stg[g % 4][:, 0:ncols],
                            ps[g % 4][0][:, 0:ncols]).then_inc(a_done, 1)

        @block.vector
        def _(vector):
            for g in range(GT):
                vector.wait_ge(m_done, g + 1)
                if g >= 4 and mode in ("full", "out"):
                    vector.wait_ge(o_sem, 16 * (g - 3))
                vector.tensor_copy(
                    out=stg[g % 4][:, 512:512 + ncols],
                    in_=ps[g % 4][1][:, 0:ncols]).then_inc(v_done, 1)

        if mode in ("full", "out"):
            @block.gpsimd
            def _(gp):
                for g in range(GT):
                    gp.wait_ge(a_done, g + 1)
                    gp.wait_ge(v_done, g + 1)
                    lim = g if deep[g % G] else g - 1
                    if lim >= 1:
                        gp.wait_ge(o_sem, 16 * lim)
                    gp.indirect_dma_start(
                        out=img[:, :],
                        out_offset=bass.IndirectOffsetOnAxis(
                            ap=offs_sb[:, (g % G):(g % G) + 1], axis=0),
                        in_=stg[g % 4][:, :],
                        in_offset=None,
                        compute_op=mybir.AluOpType.add,
                    ).then_inc(o_sem, 16)

    return nc


# ---------------------------------------------------------------- host side


def make_in_maps(sensor_data, plan):
    _, _, blocks_all = _tables()
    if USE_BF16:
        import ml_dtypes
        if "BLOCKS16" not in _CONST:
            _CONST["BLOCKS16"] = blocks_all.astype(ml_dtypes.bfloat16)
        blocks_all = _CONST["BLOCKS16"]
    sd = np.asarray(sensor_data, np.float32)          # (4,2,128,2048)
    tr = sd.reshape(8, S, T)                           # (c, s, T)
    tr = tr[:, plan["order"], :]                       # rank order
    # data[tau', chunk*1024 + rank*8 + c]
    d = tr.reshape(8, S, NCHUNK, 128)                  # c, rank, chunk, tau'
    d = d.transpose(3, 2, 1, 0).reshape(128, NCHUNK * S * 8).copy()
    if USE_BF16:
        import ml_dtypes
        d = d.astype(ml_dtypes.bfloat16)
    in_maps = []
    for ci in range(NCORES):
        pc = plan["cores"][ci]
        uidx = np.array(pc["u_list"]) + 511
        blk = blocks_all[uidx].reshape(len(uidx) * 128, NRHO)
        offs = np.stack([g[2] for g in pc["groups"]], axis=1)  # [128, G]
        in_maps.append({"data": d, "blocks": blk, "offs": offs})
    return in_maps


# ---------------------------------------------------------------- runner


class _CoreRunner:
    """Execute one prebuilt single-core Bass module via PJRT with a cached
    jitted callable (compiles once, reruns cheaply)."""

    def __init__(self, nc, device):
        import jax
        from concourse.bass2jax import (
            _bass_exec_p, install_neuronx_cc_hook, partition_id_tensor,
        )

        install_neuronx_cc_hook()
        self.device = device
        partition_name = (
            nc.partition_id_tensor.name if nc.partition_id_tensor else None
        )
        in_names, out_names, out_avals, zero_shapes = [], [], [], []
        for alloc in nc.m.functions[0].allocations:
            if not isinstance(alloc, mybir.MemoryLocationSet):
                continue
            name = alloc.memorylocations[0].name
            if alloc.kind == "ExternalInput":
                if name != partition_name:
                    in_names.append(name)
            elif alloc.kind == "ExternalOutput":
                shape = tuple(alloc.tensor_shape)
                dtype = mybir.dt.np(alloc.dtype)
                out_names.append(name)
                out_avals.append(jax.core.ShapedArray(shape, dtype))
                zero_shapes.append((shape, dtype))
        self.in_names = in_names
        self.out_names = out_names
        self.zero_shapes = zero_shapes
        n_params = len(in_names)
        n_outs = len(out_avals)
        all_in_names = in_names + out_names
        if partition_name is not None:
            all_in_names.append(partition_name)
        donate = tuple(range(n_params, n_params + n_outs))

        def _body(*args):
            operands = list(args)
            if partition_name is not None:
                operands.append(partition_id_tensor())
            outs = _bass_exec_p.bind(
                *operands,
                out_avals=tuple(out_avals),
                in_names=tuple(all_in_names),
                out_names=tuple(out_names),
                lowering_input_output_aliases=(),
                sim_require_finite=True,
                sim_require_nnan=True,
                nc=nc,
            )
            return tuple(outs)

        self._fn = jax.jit(_body, donate_argnums=donate, keep_unused=True)

    def launch(self, in_map):
        import jax

        cin = [jax.device_put(np.asarray(in_map[n]), self.device)
               for n in self.in_names]
        zouts = [jax.device_put(np.zeros(s, d), self.device)
                 for (s, d) in self.zero_shapes]
        return self._fn(*cin, *zouts)


_RUN_CACHE = {}


def _get_runners(plan_key, plan, repeat=1):
    import jax

    key = (plan_key, repeat)
    if key not in _RUN_CACHE:
        devs = jax.devices()
        runners = []
        for ci in range(NCORES):
            nc = build_core_nc(plan["cores"][ci], repeat)
            runners.append(_CoreRunner(nc, devs[ci]))
        _RUN_CACHE[key] = runners
    return _RUN_CACHE[key]


def kernel(sensor_data, sensor_xy):
    plan_key = np.asarray(sensor_xy).tobytes()
    plan = make_plan(sensor_xy)
    in_maps = make_in_maps(sensor_data, plan)
    runners = _get_runners(plan_key, plan)

    outs = [runners[ci].launch(in_maps[ci]) for ci in range(NCORES)]
    acc = np.zeros((8, 512, 512), np.float64)
    for ci in range(NCORES):
        img = np.asarray(outs[ci][0]).reshape(NROW + 128, PADW)
        acc += img[:NROW, 511:1023].reshape(8, 512, 512)
    return acc.astype(np.float32).reshape(4, 2, 512, 512)
